# revision 2
# baseline (speedup 1.0000x reference)
"""Trainium2 Bass kernel for nn_CrAKNVectorAttention (N=1024, C=256, 8 cores).

Math: the reference computes
    w   = softmax(h, axis=-2)                  # over j
    out[i,k] = (sum_j w[i,j,k]) * v[i,k]
and sum_j softmax_j(...) == 1 exactly, so the whole [N,C,C] relation cube
(q/k projections, LayerNorms, Mish, weight_encoding MLP) cancels out:
    out = v = feat @ Wv + bv
(verified numerically: ~4.5e-7 relative deviation, pure fp32 rounding in
the softmax normalization).

Sharding: data-parallel over N across 8 cores (128 rows each); Wv/bv
replicated.  Per core the kernel computes outT = Wv.T @ featT + bv in two
output-channel chunks:

  - Inputs arrive as one packed [128, 770] buffer sliced by FOUR parallel
    DMAs — three on SP (HWDGE) + one on Pool (SWDGE) — hoisted ahead of the
    Bass preamble barrier so the first transfer starts at t=0.  Slices are
    ordered so the PE never stalls: each of the 4 matmuls' operands land
    just before it is issued.
  - 4 matmuls (fp32, K=128 chunks) accumulate into two PSUM banks.
  - DVE evicts each PSUM chunk with a fused per-partition bias add
    (tensor_scalar_add).
  - Output ships via two pre-armed SWDGE scatters (descriptors generated
    early on Pool with iota-built indices; a cheap trigger_dma fires each
    as its eviction lands) — skipping the HWDGE + DGE-delay latency of a
    normal store DMA.  The scatter adds into the runtime-pre-zeroed output
    buffer, padded to 384 rows so the index tensor can span all 128
    partitions (only partitions 0..15 are semantically read; the host
    discards rows 256..383).

Falls back to a plain Tile-based version of the same math if the
aggressive path fails to build or run in the target environment.
"""

import numpy as np

N, C = 1024, 256
N_CORES = 8
ROWS = N // N_CORES  # 128
P = 128
W_PK = 770

_CACHE = {}


def _build_fast():
    import concourse.bacc as bacc
    import concourse.mybir as mybir

    f32 = mybir.dt.float32
    i16 = mybir.dt.int16
    nc = bacc.Bacc("TRN2", target_bir_lowering=False, debug=False,
                   num_devices=N_CORES)

    pk_d = nc.dram_tensor("pk", [P, W_PK], f32, kind="ExternalInput").ap()
    # padded to 384 rows: iota-generated scatter indices from unused SBUF
    # partitions (16..127) land in rows [256:384), which the host discards
    out_d = nc.dram_tensor("outT", [C + P, ROWS], f32,
                           kind="ExternalOutput").ap()

    n_pre = len(nc.main_func.blocks[0].instructions)

    with (
        nc.sbuf_tensor([P, 256], f32) as A_t,
        nc.sbuf_tensor([P, 128], f32) as B_t,
        nc.sbuf_tensor([P, 256], f32) as C_t,
        nc.sbuf_tensor([P, 130], f32) as D_t,
        nc.sbuf_tensor([P, 2 * ROWS], f32) as ot_t,
        nc.sbuf_tensor([P, 16], i16) as idx_t,
        nc.psum_tensor([P, 512], f32) as psb0,
        nc.psum_tensor([P, 512], f32) as psb1,
        nc.semaphore() as d1,
        nc.semaphore() as dp,
        nc.semaphore() as d2,
        nc.semaphore() as d3,
        nc.semaphore() as pes,
        nc.semaphore() as v0,
        nc.semaphore() as v1,
        nc.semaphore() as prep_sem,
        nc.semaphore() as dout,
    ):
        A = A_t.ap()
        Bt = B_t.ap()
        Ct = C_t.ap()
        D = D_t.ap()
        ot = ot_t.ap()
        idx = idx_t.ap()
        ps0 = psb0.ap()[:, 0:ROWS]
        ps1 = psb1.ap()[:, 0:ROWS]

        # input DMAs (hoisted to t=0)
        nc.sync.dma_start(A[:], pk_d[:, 0:256]).then_inc(d1, 16)
        nc.sync.dma_start(Ct[:], pk_d[:, 384:640]).then_inc(d2, 16)
        nc.sync.dma_start(D[:], pk_d[:, 640:770]).then_inc(d3, 16)
        nc.gpsimd.dma_start(Bt[:], pk_d[:, 256:384]).then_inc(dp, 16)

        # Pool: scatter indices + pre-armed output scatters
        nc.gpsimd.iota(idx[:, 0:8], [[16, 8]], base=0, channel_multiplier=1)
        nc.gpsimd.iota(idx[:, 8:16], [[16, 8]], base=128, channel_multiplier=1)
        nc.gpsimd.dma_scatter_add(
            out_d[:, :], ot[:, 0:ROWS].rearrange("p (g m) -> p g m", g=1),
            idx[:, 0:8], ROWS, ROWS, ROWS,
            prepare_only=True, sem=dout).then_inc(prep_sem, 1)
        nc.gpsimd.dma_scatter_add(
            out_d[:, :], ot[:, ROWS:2 * ROWS].rearrange("p (g m) -> p g m", g=1),
            idx[:, 8:16], ROWS, ROWS, ROWS,
            prepare_only=True, sem=dout).then_inc(prep_sem, 1)

        # PE: 4 matmuls in chunk-arrival order
        nc.tensor.wait_ge(d1, 16)
        nc.tensor.matmul(ps0, A[:, 128:256], A[:, 0:128], start=True, stop=False)
        nc.tensor.wait_ge(dp, 16)
        nc.tensor.matmul(ps1, Bt[:, 0:128], A[:, 0:128], start=True, stop=False)
        nc.tensor.wait_ge(d2, 16)
        nc.tensor.matmul(ps0, Ct[:, 128:256], Ct[:, 0:128],
                         start=False, stop=True).then_inc(pes, 1)
        nc.tensor.wait_ge(d3, 16)
        nc.tensor.matmul(ps1, D[:, 0:128], Ct[:, 0:128],
                         start=False, stop=True).then_inc(pes, 1)

        # DVE: per-chunk eviction with fused bias
        nc.vector.wait_ge(pes, 1)
        nc.vector.tensor_scalar_add(
            ot[:, 0:ROWS], ps0, D[:, 128:129]).then_inc(v0, 1)
        nc.vector.wait_ge(pes, 2)
        nc.vector.tensor_scalar_add(
            ot[:, ROWS:2 * ROWS], ps1,
            D[:, 129:130]).then_inc(v1, 1)

        # Pool: fire scatters as evictions land
        nc.gpsimd.wait_ge(prep_sem, 1)
        nc.gpsimd.wait_ge(v0, 1)
        nc.gpsimd.trigger_dma(count=1)
        nc.gpsimd.wait_ge(prep_sem, 2)
        nc.gpsimd.wait_ge(v1, 1)
        nc.gpsimd.trigger_dma(count=1)
        nc.gpsimd.wait_ge(dout, 32)

        # hoist the input DMAs and index generation ahead of the Bass
        # preamble (const memsets + all-engine barrier): they touch only
        # our tiles, and the preamble barrier otherwise delays the first
        # transfer by ~650ns
        insts = nc.main_func.blocks[0].instructions
        moved = [i for i in insts[n_pre:]
                 if type(i).__name__ == "InstDMACopy"
                 and i.engine in (mybir.EngineType.SP,
                                  mybir.EngineType.Pool)][:4]
        moved += [i for i in insts[n_pre:]
                  if type(i).__name__ == "InstIota"][:2]
        for m in moved:
            insts.remove(m)
        for m in reversed(moved):
            insts.insert(0, m)

    nc.compile()
    return nc


def _build_fallback():
    """Plain Tile version: 2-way split packed input, 4 matmuls, DVE
    bias-add eviction, single output DMA, transposed output layout."""
    import concourse.bacc as bacc
    import concourse.bass as bass
    import concourse.mybir as mybir
    from concourse import tile

    f32 = mybir.dt.float32
    nc = bacc.Bacc("TRN2", target_bir_lowering=False, debug=False,
                   num_devices=N_CORES)

    pk_d = nc.dram_tensor("pk", [P, W_PK], f32, kind="ExternalInput").ap()
    out_d = nc.dram_tensor("outT", [C + P, ROWS], f32,
                           kind="ExternalOutput").ap()

    with tile.TileContext(nc) as tc:
        with (
            tc.tile_pool(name="sbuf", bufs=1) as pool,
            tc.tile_pool(name="psum", bufs=1, space=bass.MemorySpace.PSUM) as pp,
        ):
            pkA = pool.tile([P, 384], f32)   # ftA | wvA_a0 | wvA_a1
            pkB = pool.tile([P, 386], f32)   # ftB | wvB_a0 | wvB_a1 | bias
            ps0 = pp.tile([P, ROWS], f32, name="ps0")
            ps1 = pp.tile([P, ROWS], f32, name="ps1")
            ot = pool.tile([P, 2 * ROWS], f32)

            nc.sync.dma_start(pkA[:], pk_d[:, 0:384])
            nc.sync.dma_start(pkB[:], pk_d[:, 384:770])

            nc.tensor.matmul(ps0[:], pkA[:, 128:256], pkA[:, 0:128],
                             start=True, stop=False)
            nc.tensor.matmul(ps1[:], pkA[:, 256:384], pkA[:, 0:128],
                             start=True, stop=False)
            nc.tensor.matmul(ps0[:], pkB[:, 128:256], pkB[:, 0:128],
                             start=False, stop=True)
            nc.tensor.matmul(ps1[:], pkB[:, 256:384], pkB[:, 0:128],
                             start=False, stop=True)

            nc.vector.tensor_scalar_add(ot[:, 0:ROWS], ps0[:],
                                        pkB[:, 384:385])
            nc.vector.tensor_scalar_add(ot[:, ROWS:2 * ROWS], ps1[:],
                                        pkB[:, 385:386])

            nc.sync.dma_start(
                out_d[0:C].rearrange("(a p) m -> p a m", a=2),
                ot.rearrange("p (a m) -> p a m", a=2))

    nc.compile()
    return nc


def pack_inputs(feat, Wv, bv):
    feat = np.asarray(feat, dtype=np.float32)
    Wv = np.ascontiguousarray(np.asarray(Wv, dtype=np.float32))
    bv = np.asarray(bv, dtype=np.float32).reshape(C)
    bt = bv.reshape(2, P).T  # [P, 2]; col a holds bv[a*128 + p]
    maps = []
    for c in range(N_CORES):
        ftT = feat[c * ROWS:(c + 1) * ROWS, :].T  # [C, ROWS]
        pk = np.empty((P, W_PK), np.float32)
        pk[:, 0:128] = ftT[0:P, :]            # ftA
        pk[:, 128:256] = Wv[0:P, 0:128]       # wvA_a0
        pk[:, 256:384] = Wv[0:P, 128:256]     # wvA_a1
        pk[:, 384:512] = ftT[P:C, :]          # ftB
        pk[:, 512:640] = Wv[P:C, 0:128]       # wvB_a0
        pk[:, 640:768] = Wv[P:C, 128:256]     # wvB_a1
        pk[:, 768:770] = bt                   # bias
        maps.append({"pk": pk})
    return maps


def _get_nc():
    if "nc" not in _CACHE:
        try:
            _CACHE["nc"] = _build_fast()
        except Exception:
            _CACHE["nc"] = _build_fallback()
    return _CACHE["nc"]


def _run(inputs, **run_kwargs):
    from concourse.bass_utils import run_bass_kernel_spmd

    nc = _get_nc()
    in_maps = pack_inputs(inputs["feat"], inputs["Wv"], inputs["bv"])
    res = run_bass_kernel_spmd(nc, in_maps, list(range(N_CORES)), **run_kwargs)
    parts = [np.ascontiguousarray(res.results[c]["outT"][0:C].T)
             for c in range(N_CORES)]
    return np.concatenate(parts, axis=0), res


def kernel(**inputs) -> np.ndarray:
    try:
        out, _ = _run(inputs)
    except Exception:
        if "nc" in _CACHE:  # retry once on the fallback program
            del _CACHE["nc"]
            _CACHE["nc"] = _build_fallback()
            out, _ = _run(inputs)
        else:
            raise
    return out


# revision 3
# speedup vs baseline: 1.0104x; 1.0104x over previous
"""Trainium2 Bass kernel for nn_CrAKNVectorAttention (N=1024, C=256, 8 cores).

Math: the reference computes
    w   = softmax(h, axis=-2)                  # over j
    out[i,k] = (sum_j w[i,j,k]) * v[i,k]
and sum_j softmax_j(...) == 1 exactly, so the whole [N,C,C] relation cube
(q/k projections, LayerNorms, Mish, weight_encoding MLP) cancels out:
    out = v = feat @ Wv + bv
(verified numerically: ~4.5e-7 relative deviation, pure fp32 rounding in
the softmax normalization).

Sharding: data-parallel over N across 8 cores (128 rows each); Wv/bv
replicated.  Per core the kernel computes outT = Wv.T @ featT + bv in two
output-channel chunks:

  - Inputs arrive as one packed [128, 770] buffer sliced by FOUR parallel
    DMAs — three on SP (HWDGE) + one on Pool (SWDGE) — hoisted ahead of the
    Bass preamble barrier so the first transfer starts at t=0.  Slices are
    ordered so the PE never stalls: each of the 4 matmuls' operands land
    just before it is issued.
  - 4 matmuls (fp32, K=128 chunks) accumulate into two PSUM banks.
  - DVE evicts each PSUM chunk with a fused per-partition bias add
    (tensor_scalar_add).
  - Output ships via two pre-armed SWDGE scatters (descriptors generated
    early on Pool with iota-built indices; a cheap trigger_dma fires each
    as its eviction lands) — skipping the HWDGE + DGE-delay latency of a
    normal store DMA.  The scatter adds into the runtime-pre-zeroed output
    buffer, padded to 384 rows so the index tensor can span all 128
    partitions (only partitions 0..15 are semantically read; the host
    discards rows 256..383).

Falls back to a plain Tile-based version of the same math if the
aggressive path fails to build or run in the target environment.
"""

import numpy as np

N, C = 1024, 256
N_CORES = 8
ROWS = N // N_CORES  # 128
P = 128
W_PK = 770

_CACHE = {}


def _build_fast():
    import concourse.bacc as bacc
    import concourse.mybir as mybir

    f32 = mybir.dt.float32
    i16 = mybir.dt.int16
    nc = bacc.Bacc("TRN2", target_bir_lowering=False, debug=False,
                   num_devices=N_CORES)

    pk_d = nc.dram_tensor("pk", [P, W_PK], f32, kind="ExternalInput").ap()
    # padded to 384 rows: iota-generated scatter indices from unused SBUF
    # partitions (16..127) land in rows [256:384), which the host discards
    out_d = nc.dram_tensor("outT", [C + P, ROWS], f32,
                           kind="ExternalOutput").ap()

    n_pre = len(nc.main_func.blocks[0].instructions)

    with (
        nc.sbuf_tensor([P, 256], f32) as A_t,
        nc.sbuf_tensor([P, 128], f32) as B_t,
        nc.sbuf_tensor([P, 256], f32) as C_t,
        nc.sbuf_tensor([P, 130], f32) as D_t,
        nc.sbuf_tensor([P, 2 * ROWS], f32) as ot_t,
        nc.sbuf_tensor([P, 16], i16) as idx_t,
        nc.psum_tensor([P, 512], f32) as psb0,
        nc.psum_tensor([P, 512], f32) as psb1,
        nc.semaphore() as d1,
        nc.semaphore() as dp,
        nc.semaphore() as d2,
        nc.semaphore() as d3,
        nc.semaphore() as pes,
        nc.semaphore() as v0,
        nc.semaphore() as v1,
        nc.semaphore() as prep_sem,
        nc.semaphore() as dout,
    ):
        A = A_t.ap()
        Bt = B_t.ap()
        Ct = C_t.ap()
        D = D_t.ap()
        ot = ot_t.ap()
        idx = idx_t.ap()
        ps0 = psb0.ap()[:, 0:ROWS]
        ps1 = psb1.ap()[:, 0:ROWS]

        # input DMAs (hoisted to t=0)
        nc.sync.dma_start(A[:], pk_d[:, 0:256]).then_inc(d1, 16)
        nc.sync.dma_start(Ct[:], pk_d[:, 384:640]).then_inc(d2, 16)
        nc.sync.dma_start(D[:], pk_d[:, 640:770]).then_inc(d3, 16)
        nc.gpsimd.dma_start(Bt[:], pk_d[:, 256:384]).then_inc(dp, 16)

        # Pool: scatter indices + pre-armed output scatters
        nc.gpsimd.iota(idx[:, 0:8], [[16, 8]], base=0, channel_multiplier=1)
        nc.gpsimd.iota(idx[:, 8:16], [[16, 8]], base=128, channel_multiplier=1)
        nc.gpsimd.dma_scatter_add(
            out_d[:, :], ot[:, 0:ROWS].rearrange("p (g m) -> p g m", g=1),
            idx[:, 0:8], ROWS, ROWS, ROWS,
            prepare_only=True, sem=dout).then_inc(prep_sem, 1)
        nc.gpsimd.dma_scatter_add(
            out_d[:, :], ot[:, ROWS:2 * ROWS].rearrange("p (g m) -> p g m", g=1),
            idx[:, 8:16], ROWS, ROWS, ROWS,
            prepare_only=True, sem=dout).then_inc(prep_sem, 1)

        # PE: 4 matmuls in chunk-arrival order
        nc.tensor.wait_ge(d1, 16)
        nc.tensor.matmul(ps0, A[:, 128:256], A[:, 0:128], start=True, stop=False)
        nc.tensor.wait_ge(dp, 16)
        nc.tensor.matmul(ps1, Bt[:, 0:128], A[:, 0:128], start=True, stop=False)
        nc.tensor.wait_ge(d2, 16)
        nc.tensor.matmul(ps0, Ct[:, 128:256], Ct[:, 0:128],
                         start=False, stop=True).then_inc(pes, 1)
        nc.tensor.wait_ge(d3, 16)
        nc.tensor.matmul(ps1, D[:, 0:128], Ct[:, 0:128],
                         start=False, stop=True).then_inc(pes, 1)

        # DVE: per-chunk eviction with fused bias
        nc.vector.wait_ge(pes, 1)
        nc.vector.tensor_scalar_add(
            ot[:, 0:ROWS], ps0, D[:, 128:129]).then_inc(v0, 1)
        nc.vector.wait_ge(pes, 2)
        nc.vector.tensor_scalar_add(
            ot[:, ROWS:2 * ROWS], ps1,
            D[:, 129:130]).then_inc(v1, 1)

        # Pool: fire scatters as evictions land
        nc.gpsimd.wait_ge(prep_sem, 1)
        t0 = nc.gpsimd.trigger_dma(count=1)
        t0._wait_ge(v0, 1)
        nc.gpsimd.wait_ge(prep_sem, 2)
        t1 = nc.gpsimd.trigger_dma(count=1)
        t1._wait_ge(v1, 1)
        nc.gpsimd.wait_ge(dout, 32)

        # hoist the input DMAs and index generation ahead of the Bass
        # preamble (const memsets + all-engine barrier): they touch only
        # our tiles, and the preamble barrier otherwise delays the first
        # transfer by ~650ns
        insts = nc.main_func.blocks[0].instructions
        moved = [i for i in insts[n_pre:]
                 if type(i).__name__ == "InstDMACopy"
                 and i.engine in (mybir.EngineType.SP,
                                  mybir.EngineType.Pool)][:4]
        moved += [i for i in insts[n_pre:]
                  if type(i).__name__ == "InstIota"][:2]
        for m in moved:
            insts.remove(m)
        for m in reversed(moved):
            insts.insert(0, m)

    nc.compile()
    return nc


def _build_fallback():
    """Plain Tile version: 2-way split packed input, 4 matmuls, DVE
    bias-add eviction, single output DMA, transposed output layout."""
    import concourse.bacc as bacc
    import concourse.bass as bass
    import concourse.mybir as mybir
    from concourse import tile

    f32 = mybir.dt.float32
    nc = bacc.Bacc("TRN2", target_bir_lowering=False, debug=False,
                   num_devices=N_CORES)

    pk_d = nc.dram_tensor("pk", [P, W_PK], f32, kind="ExternalInput").ap()
    out_d = nc.dram_tensor("outT", [C + P, ROWS], f32,
                           kind="ExternalOutput").ap()

    with tile.TileContext(nc) as tc:
        with (
            tc.tile_pool(name="sbuf", bufs=1) as pool,
            tc.tile_pool(name="psum", bufs=1, space=bass.MemorySpace.PSUM) as pp,
        ):
            pkA = pool.tile([P, 384], f32)   # ftA | wvA_a0 | wvA_a1
            pkB = pool.tile([P, 386], f32)   # ftB | wvB_a0 | wvB_a1 | bias
            ps0 = pp.tile([P, ROWS], f32, name="ps0")
            ps1 = pp.tile([P, ROWS], f32, name="ps1")
            ot = pool.tile([P, 2 * ROWS], f32)

            nc.sync.dma_start(pkA[:], pk_d[:, 0:384])
            nc.sync.dma_start(pkB[:], pk_d[:, 384:770])

            nc.tensor.matmul(ps0[:], pkA[:, 128:256], pkA[:, 0:128],
                             start=True, stop=False)
            nc.tensor.matmul(ps1[:], pkA[:, 256:384], pkA[:, 0:128],
                             start=True, stop=False)
            nc.tensor.matmul(ps0[:], pkB[:, 128:256], pkB[:, 0:128],
                             start=False, stop=True)
            nc.tensor.matmul(ps1[:], pkB[:, 256:384], pkB[:, 0:128],
                             start=False, stop=True)

            nc.vector.tensor_scalar_add(ot[:, 0:ROWS], ps0[:],
                                        pkB[:, 384:385])
            nc.vector.tensor_scalar_add(ot[:, ROWS:2 * ROWS], ps1[:],
                                        pkB[:, 385:386])

            nc.sync.dma_start(
                out_d[0:C].rearrange("(a p) m -> p a m", a=2),
                ot.rearrange("p (a m) -> p a m", a=2))

    nc.compile()
    return nc


def pack_inputs(feat, Wv, bv):
    feat = np.asarray(feat, dtype=np.float32)
    Wv = np.ascontiguousarray(np.asarray(Wv, dtype=np.float32))
    bv = np.asarray(bv, dtype=np.float32).reshape(C)
    bt = bv.reshape(2, P).T  # [P, 2]; col a holds bv[a*128 + p]
    maps = []
    for c in range(N_CORES):
        ftT = feat[c * ROWS:(c + 1) * ROWS, :].T  # [C, ROWS]
        pk = np.empty((P, W_PK), np.float32)
        pk[:, 0:128] = ftT[0:P, :]            # ftA
        pk[:, 128:256] = Wv[0:P, 0:128]       # wvA_a0
        pk[:, 256:384] = Wv[0:P, 128:256]     # wvA_a1
        pk[:, 384:512] = ftT[P:C, :]          # ftB
        pk[:, 512:640] = Wv[P:C, 0:128]       # wvB_a0
        pk[:, 640:768] = Wv[P:C, 128:256]     # wvB_a1
        pk[:, 768:770] = bt                   # bias
        maps.append({"pk": pk})
    return maps


def _get_nc():
    if "nc" not in _CACHE:
        try:
            _CACHE["nc"] = _build_fast()
        except Exception:
            _CACHE["nc"] = _build_fallback()
    return _CACHE["nc"]


def _run(inputs, **run_kwargs):
    from concourse.bass_utils import run_bass_kernel_spmd

    nc = _get_nc()
    in_maps = pack_inputs(inputs["feat"], inputs["Wv"], inputs["bv"])
    res = run_bass_kernel_spmd(nc, in_maps, list(range(N_CORES)), **run_kwargs)
    parts = [np.ascontiguousarray(res.results[c]["outT"][0:C].T)
             for c in range(N_CORES)]
    return np.concatenate(parts, axis=0), res


def kernel(**inputs) -> np.ndarray:
    try:
        out, _ = _run(inputs)
    except Exception:
        if "nc" in _CACHE:  # retry once on the fallback program
            del _CACHE["nc"]
            _CACHE["nc"] = _build_fallback()
            out, _ = _run(inputs)
        else:
            raise
    return out


# revision 4
# speedup vs baseline: 1.0118x; 1.0014x over previous
"""Trainium2 Bass kernel for nn_CrAKNVectorAttention (N=1024, C=256, 8 cores).

Math: the reference computes
    w   = softmax(h, axis=-2)                  # over j
    out[i,k] = (sum_j w[i,j,k]) * v[i,k]
and sum_j softmax_j(...) == 1 exactly, so the whole [N,C,C] relation cube
(q/k projections, LayerNorms, Mish, weight_encoding MLP) cancels out:
    out = v = feat @ Wv + bv
(verified numerically: ~4.5e-7 relative deviation, pure fp32 rounding in
the softmax normalization).

Sharding: data-parallel over N across 8 cores (128 rows each); Wv/bv
replicated.  Per core the kernel computes outT = Wv.T @ featT + bv in two
output-channel chunks:

  - Inputs arrive as one packed [128, 770] buffer sliced by FOUR parallel
    DMAs — three on SP (HWDGE) + one on Pool (SWDGE) — hoisted ahead of the
    Bass preamble barrier so the first transfer starts at t=0.  Slices are
    ordered so the PE never stalls: each of the 4 matmuls' operands land
    just before it is issued.
  - 4 matmuls (fp32, K=128 chunks) accumulate into two PSUM banks.
  - DVE evicts each PSUM chunk with a fused per-partition bias add
    (tensor_scalar_add).
  - Output ships via two pre-armed SWDGE scatters (descriptors generated
    early on Pool with iota-built indices; a cheap trigger_dma fires each
    as its eviction lands) — skipping the HWDGE + DGE-delay latency of a
    normal store DMA.  The scatter adds into the runtime-pre-zeroed output
    buffer, padded to 384 rows so the index tensor can span all 128
    partitions (only partitions 0..15 are semantically read; the host
    discards rows 256..383).

Falls back to a plain Tile-based version of the same math if the
aggressive path fails to build or run in the target environment.
"""

import numpy as np

N, C = 1024, 256
N_CORES = 8
ROWS = N // N_CORES  # 128
P = 128
W_PK = 770

_CACHE = {}


def _build_fast():
    import concourse.bacc as bacc
    import concourse.mybir as mybir

    f32 = mybir.dt.float32
    i16 = mybir.dt.int16
    nc = bacc.Bacc("TRN2", target_bir_lowering=False, debug=False,
                   num_devices=N_CORES)

    pk_d = nc.dram_tensor("pk", [P, W_PK], f32, kind="ExternalInput").ap()
    # padded to 384 rows: iota-generated scatter indices from unused SBUF
    # partitions (16..127) land in rows [256:384), which the host discards
    out_d = nc.dram_tensor("outT", [C + P, ROWS], f32,
                           kind="ExternalOutput").ap()

    n_pre = len(nc.main_func.blocks[0].instructions)

    with (
        nc.sbuf_tensor([P, 256], f32) as A_t,
        nc.sbuf_tensor([P, 128], f32) as B_t,
        nc.sbuf_tensor([P, 256], f32) as C_t,
        nc.sbuf_tensor([P, 130], f32) as D_t,
        nc.sbuf_tensor([P, 2 * ROWS], f32) as ot_t,
        nc.sbuf_tensor([P, 16], i16) as idx_t,
        nc.psum_tensor([P, 512], f32) as psb0,
        nc.psum_tensor([P, 512], f32) as psb1,
        nc.semaphore() as d1,
        nc.semaphore() as dp,
        nc.semaphore() as d2,
        nc.semaphore() as d3,
        nc.semaphore() as pes,
        nc.semaphore() as v0,
        nc.semaphore() as v1,
        nc.semaphore() as prep_sem,
        nc.semaphore() as dout,
    ):
        A = A_t.ap()
        Bt = B_t.ap()
        Ct = C_t.ap()
        D = D_t.ap()
        ot = ot_t.ap()
        idx = idx_t.ap()
        ps0 = psb0.ap()[:, 0:ROWS]
        ps1 = psb1.ap()[:, 0:ROWS]

        # input DMAs (hoisted to t=0)
        nc.sync.dma_start(A[:], pk_d[:, 0:256]).then_inc(d1, 16)
        nc.sync.dma_start(Ct[:], pk_d[:, 384:640]).then_inc(d2, 16)
        nc.sync.dma_start(D[:], pk_d[:, 640:770]).then_inc(d3, 16)
        nc.gpsimd.dma_start(Bt[:], pk_d[:, 256:384]).then_inc(dp, 16)

        # Pool: scatter indices + pre-armed output scatters
        nc.gpsimd.iota(idx[:, 0:8], [[16, 8]], base=0, channel_multiplier=1)
        nc.gpsimd.iota(idx[:, 8:16], [[16, 8]], base=128, channel_multiplier=1)
        nc.gpsimd.dma_scatter_add(
            out_d[:, :], ot[:, 0:ROWS].rearrange("p (g m) -> p g m", g=1),
            idx[:, 0:8], ROWS, ROWS, ROWS,
            prepare_only=True, sem=dout).then_inc(prep_sem, 1)
        nc.gpsimd.dma_scatter_add(
            out_d[:, :], ot[:, ROWS:2 * ROWS].rearrange("p (g m) -> p g m", g=1),
            idx[:, 8:16], ROWS, ROWS, ROWS,
            prepare_only=True, sem=dout).then_inc(prep_sem, 1)

        # PE: 4 matmuls in chunk-arrival order
        nc.tensor.wait_ge(d1, 16)
        nc.tensor.matmul(ps0, A[:, 128:256], A[:, 0:128], start=True, stop=False)
        nc.tensor.wait_ge(dp, 16)
        nc.tensor.matmul(ps1, Bt[:, 0:128], A[:, 0:128], start=True, stop=False)
        nc.tensor.wait_ge(d2, 16)
        nc.tensor.matmul(ps0, Ct[:, 128:256], Ct[:, 0:128],
                         start=False, stop=True).then_inc(pes, 1)
        nc.tensor.wait_ge(d3, 16)
        nc.tensor.matmul(ps1, D[:, 0:128], Ct[:, 0:128],
                         start=False, stop=True).then_inc(pes, 1)

        # DVE: per-chunk eviction with fused bias
        nc.vector.wait_ge(pes, 1)
        nc.vector.tensor_scalar_add(
            ot[:, 0:ROWS], ps0, D[:, 128:129]).then_inc(v0, 1)
        nc.vector.wait_ge(pes, 2)
        nc.vector.tensor_scalar_add(
            ot[:, ROWS:2 * ROWS], ps1,
            D[:, 129:130]).then_inc(v1, 1)

        # Pool: fire scatters as evictions land
        nc.gpsimd.wait_ge(prep_sem, 1)
        t0 = nc.gpsimd.trigger_dma(count=1)
        t0._wait_ge(v0, 1)
        nc.gpsimd.wait_ge(prep_sem, 2)
        t1 = nc.gpsimd.trigger_dma(count=1)
        t1._wait_ge(v1, 1)
        nc.sync.wait_ge(dout, 32)

        # hoist the input DMAs and index generation ahead of the Bass
        # preamble (const memsets + all-engine barrier): they touch only
        # our tiles, and the preamble barrier otherwise delays the first
        # transfer by ~650ns
        insts = nc.main_func.blocks[0].instructions
        moved = [i for i in insts[n_pre:]
                 if type(i).__name__ == "InstDMACopy"
                 and i.engine in (mybir.EngineType.SP,
                                  mybir.EngineType.Pool)][:4]
        moved += [i for i in insts[n_pre:]
                  if type(i).__name__ == "InstIota"][:2]
        for m in moved:
            insts.remove(m)
        for m in reversed(moved):
            insts.insert(0, m)

    nc.compile()
    return nc


def _build_fallback():
    """Plain Tile version: 2-way split packed input, 4 matmuls, DVE
    bias-add eviction, single output DMA, transposed output layout."""
    import concourse.bacc as bacc
    import concourse.bass as bass
    import concourse.mybir as mybir
    from concourse import tile

    f32 = mybir.dt.float32
    nc = bacc.Bacc("TRN2", target_bir_lowering=False, debug=False,
                   num_devices=N_CORES)

    pk_d = nc.dram_tensor("pk", [P, W_PK], f32, kind="ExternalInput").ap()
    out_d = nc.dram_tensor("outT", [C + P, ROWS], f32,
                           kind="ExternalOutput").ap()

    with tile.TileContext(nc) as tc:
        with (
            tc.tile_pool(name="sbuf", bufs=1) as pool,
            tc.tile_pool(name="psum", bufs=1, space=bass.MemorySpace.PSUM) as pp,
        ):
            pkA = pool.tile([P, 384], f32)   # ftA | wvA_a0 | wvA_a1
            pkB = pool.tile([P, 386], f32)   # ftB | wvB_a0 | wvB_a1 | bias
            ps0 = pp.tile([P, ROWS], f32, name="ps0")
            ps1 = pp.tile([P, ROWS], f32, name="ps1")
            ot = pool.tile([P, 2 * ROWS], f32)

            nc.sync.dma_start(pkA[:], pk_d[:, 0:384])
            nc.sync.dma_start(pkB[:], pk_d[:, 384:770])

            nc.tensor.matmul(ps0[:], pkA[:, 128:256], pkA[:, 0:128],
                             start=True, stop=False)
            nc.tensor.matmul(ps1[:], pkA[:, 256:384], pkA[:, 0:128],
                             start=True, stop=False)
            nc.tensor.matmul(ps0[:], pkB[:, 128:256], pkB[:, 0:128],
                             start=False, stop=True)
            nc.tensor.matmul(ps1[:], pkB[:, 256:384], pkB[:, 0:128],
                             start=False, stop=True)

            nc.vector.tensor_scalar_add(ot[:, 0:ROWS], ps0[:],
                                        pkB[:, 384:385])
            nc.vector.tensor_scalar_add(ot[:, ROWS:2 * ROWS], ps1[:],
                                        pkB[:, 385:386])

            nc.sync.dma_start(
                out_d[0:C].rearrange("(a p) m -> p a m", a=2),
                ot.rearrange("p (a m) -> p a m", a=2))

    nc.compile()
    return nc


def pack_inputs(feat, Wv, bv):
    feat = np.asarray(feat, dtype=np.float32)
    Wv = np.ascontiguousarray(np.asarray(Wv, dtype=np.float32))
    bv = np.asarray(bv, dtype=np.float32).reshape(C)
    bt = bv.reshape(2, P).T  # [P, 2]; col a holds bv[a*128 + p]
    maps = []
    for c in range(N_CORES):
        ftT = feat[c * ROWS:(c + 1) * ROWS, :].T  # [C, ROWS]
        pk = np.empty((P, W_PK), np.float32)
        pk[:, 0:128] = ftT[0:P, :]            # ftA
        pk[:, 128:256] = Wv[0:P, 0:128]       # wvA_a0
        pk[:, 256:384] = Wv[0:P, 128:256]     # wvA_a1
        pk[:, 384:512] = ftT[P:C, :]          # ftB
        pk[:, 512:640] = Wv[P:C, 0:128]       # wvB_a0
        pk[:, 640:768] = Wv[P:C, 128:256]     # wvB_a1
        pk[:, 768:770] = bt                   # bias
        maps.append({"pk": pk})
    return maps


def _get_nc():
    if "nc" not in _CACHE:
        try:
            _CACHE["nc"] = _build_fast()
        except Exception:
            _CACHE["nc"] = _build_fallback()
    return _CACHE["nc"]


def _run(inputs, **run_kwargs):
    from concourse.bass_utils import run_bass_kernel_spmd

    nc = _get_nc()
    in_maps = pack_inputs(inputs["feat"], inputs["Wv"], inputs["bv"])
    res = run_bass_kernel_spmd(nc, in_maps, list(range(N_CORES)), **run_kwargs)
    parts = [np.ascontiguousarray(res.results[c]["outT"][0:C].T)
             for c in range(N_CORES)]
    return np.concatenate(parts, axis=0), res


def kernel(**inputs) -> np.ndarray:
    try:
        out, _ = _run(inputs)
    except Exception:
        if "nc" in _CACHE:  # retry once on the fallback program
            del _CACHE["nc"]
            _CACHE["nc"] = _build_fallback()
            out, _ = _run(inputs)
        else:
            raise
    return out
